# revision 13
# baseline (speedup 1.0000x reference)
"""BSRBF-KAN layer (LayerNorm + ReLU-base + B-spline+RBF spline matmul) on 8 trn2 cores.

Math:
  xn = LN(x) * gamma + beta
  base_out   = relu(xn) @ base_weight.T
  spline_out = (Bspline(xn) + RBF(xn)) @ spline_weight.T        (k = d*8 + j)
  out        = base_out + spline_out

Kernel strategy (data-parallel over the 16384 tokens, 2048/core):
  - Cubic B-splines on the uniform 12-knot grid are evaluated as 4th
    differences of truncated powers  r_q = (s*relu(min(x,3.5) - a_q))^3 :
        B_j(x) = sum_i [1,-4,6,-4,1]_i * r_{j+i}(x) / (6 h^3)
    The difference operator is folded into the spline weights on the host
    (W_r), so the device only computes 12 relu^3 features per d.  The
    cancellation in the fold requires fp32 features/weights -> the r-part
    matmul runs as float32r (full PE rate at N=512).
  - RBF: e_j = exp(-((x-g_j)/den)^2) computed as exp(-(x^2 - 2 g_j x)/den^2
    - (g_j/den)^2): one fused DVE op + one ACT exp per basis -> bf16.
  - Output is produced o-major per core ([512, 2048]); host transposes.
"""

import math
import numpy as np
import ml_dtypes

import concourse.bacc as bacc
import concourse.bass as bass
import concourse.tile as tile
from concourse import mybir
from concourse.bass_utils import run_bass_kernel_spmd
from concourse.masks import make_identity
from contextlib import ExitStack

F32 = mybir.dt.float32
F32R = mybir.dt.float32r
BF16 = mybir.dt.bfloat16
AF = mybir.ActivationFunctionType
OP = mybir.AluOpType

# problem constants (hardcoded per contract)
B, S, D, O = 4, 4096, 512, 512
N_CORES = 8
TOK = (B * S) // N_CORES          # 2048 tokens per core
TBLK = 512                        # tokens per accumulation block (PSUM limit)
NBLK = TOK // TBLK                # 4 blocks per core
GRID_SIZE, SPLINE_ORDER = 5, 3
GRID_MIN, GRID_MAX = -1.5, 1.5
H = (GRID_MAX - GRID_MIN) / GRID_SIZE                    # 0.6
KNOTS = [(-SPLINE_ORDER + i) * H + GRID_MIN for i in range(12)]   # a_0..a_11
# mixed truncated-power basis: B_0..3 from LEFT powers l_q=relu(a_q-z)^3 (q=0..7),
# B_4..7 from RIGHT powers r_q=relu(z-a_q)^3 (q=4..11). Caps |feature| at ~66.
FEAT_KNOTS = [(KNOTS[q], -1.0) for q in range(8)] + [(KNOTS[q], +1.0) for q in range(4, 12)]
NQ = 16                          # truncated-power features
NJ = 8                           # rbf / spline bases
SCLIP = 3.5                      # beyond all supports; B == 0 there (both sides)
S_CUBE = (1.0 / (6.0 * H ** 3)) ** (1.0 / 3.0)           # folded 1/(6h^3)
RBF_DEN = (GRID_MAX - GRID_MIN) / (NJ - 1)               # 3/7
RBF_G = [GRID_MIN + i * RBF_DEN for i in range(NJ)]
LN_EPS = 1e-5

N_KC = 4 + NQ * 4 + NJ * 4       # 84 k-chunks of 128: base, r, e


def _fold_weights(base_weight: np.ndarray, spline_weight: np.ndarray):
    """Host-side weight prep. Returns (w_b [512,512] bf16, w_r [6144,512] f32,
    w_e [4096,512] bf16), all in lhsT layout [k, o]."""
    Wjd = spline_weight.reshape(O, D, NJ).astype(np.float64)   # [o, d, j]
    c = np.array([1.0, -4.0, 6.0, -4.0, 1.0])
    w_r = np.zeros((NQ, D, O), np.float64)                     # [q, d, o]
    for q in range(8):            # left features serve B_0..B_3
        for i in range(5):
            j = q - i
            if 0 <= j <= 3:
                w_r[q] += c[i] * Wjd[:, :, j].T
    for qi, q in enumerate(range(4, 12)):   # right features serve B_4..B_7
        for i in range(5):
            j = q - i
            if 4 <= j <= 7:
                w_r[8 + qi] += c[i] * Wjd[:, :, j].T
    w_r = w_r.reshape(NQ * D, O).astype(np.float32)
    w_e = np.ascontiguousarray(Wjd.transpose(2, 1, 0)).reshape(NJ * D, O)
    w_e = w_e.astype(ml_dtypes.bfloat16)
    w_b = np.ascontiguousarray(base_weight.T).astype(ml_dtypes.bfloat16)
    return w_b, w_r, w_e


_CACHED = {}


def _build_module():
    if "nc" in _CACHED:
        return _CACHED["nc"]
    nc = bacc.Bacc("TRN2", target_bir_lowering=False, debug=False,
                   num_devices=N_CORES)
    x_d = nc.dram_tensor("x", [TOK, D], F32, kind="ExternalInput")
    wr_d = nc.dram_tensor("w_r", [NQ * D, O], F32R, kind="ExternalInput")
    we_d = nc.dram_tensor("w_e", [NJ * D, O], BF16, kind="ExternalInput")
    wb_d = nc.dram_tensor("w_b", [D, O], BF16, kind="ExternalInput")
    g_d = nc.dram_tensor("gamma", [D], F32, kind="ExternalInput")
    be_d = nc.dram_tensor("beta", [D], F32, kind="ExternalInput")
    out_d = nc.dram_tensor("out", [O, TOK], F32, kind="ExternalOutput")

    inv_den2 = 1.0 / (RBF_DEN * RBF_DEN)

    with tile.TileContext(nc) as tc, ExitStack() as ctx:
        wpool = ctx.enter_context(tc.tile_pool(name="weights", bufs=1))
        xpool = ctx.enter_context(tc.tile_pool(name="xin", bufs=2))
        lnpool = ctx.enter_context(tc.tile_pool(name="ln", bufs=2))
        stat = ctx.enter_context(tc.tile_pool(name="stat", bufs=2))
        xtp = ctx.enter_context(tc.tile_pool(name="xnT", bufs=1))
        zpool = ctx.enter_context(tc.tile_pool(name="zt", bufs=1))
        fpool = ctx.enter_context(tc.tile_pool(name="feat", bufs=3))
        spool = ctx.enter_context(tc.tile_pool(name="scratch", bufs=2))
        opool = ctx.enter_context(tc.tile_pool(name="ostage", bufs=1))
        tpsum = ctx.enter_context(tc.tile_pool(name="tpsum", bufs=2, space="PSUM"))
        opsum = ctx.enter_context(tc.tile_pool(name="opsum", bufs=1, space="PSUM"))

        # ---- resident weights ----
        wr_sb = wpool.tile([128, NQ * 4, O], F32R)
        nc.sync.dma_start(out=wr_sb, in_=wr_d.ap().rearrange("(c p) o -> p c o", p=128))
        we_sb = wpool.tile([128, NJ * 4, O], BF16)
        nc.sync.dma_start(out=we_sb, in_=we_d.ap().rearrange("(c p) o -> p c o", p=128))
        wb_sb = wpool.tile([128, 4, O], BF16)
        nc.sync.dma_start(out=wb_sb, in_=wb_d.ap().rearrange("(c p) o -> p c o", p=128))
        gam_sb = wpool.tile([128, 4], F32)
        nc.sync.dma_start(out=gam_sb, in_=g_d.ap().rearrange("(c p) -> p c", p=128))
        bet_sb = wpool.tile([128, 4], F32)
        nc.sync.dma_start(out=bet_sb, in_=be_d.ap().rearrange("(c p) -> p c", p=128))
        ident = wpool.tile([128, 128], F32)
        make_identity(nc, ident)

        # ACT bias constants must live in SBUF ([128,1] per-partition APs)
        bias_vals = ([LN_EPS]
                     + [-sgn * a * S_CUBE for (a, sgn) in FEAT_KNOTS]
                     + [-(RBF_G[j] ** 2) * inv_den2 for j in range(NJ)])
        consts = wpool.tile([128, len(bias_vals)], F32)
        for i, v in enumerate(bias_vals):
            nc.gpsimd.memset(consts[:, i:i + 1], v)
        c_eps = consts[:, 0:1]
        c_knot = [consts[:, 1 + q:2 + q] for q in range(NQ)]
        c_rbf = [consts[:, 1 + NQ + j:2 + NQ + j] for j in range(NJ)]

        for bi in range(NBLK):
            # ---- LayerNorm (token-major) + transpose to [d, tok] ----
            xnt = [xtp.tile([128, TBLK], F32, tag=f"xnt{dt}", name=f"xnt{dt}") for dt in range(4)]
            for tt in range(TBLK // 128):
                x_t = xpool.tile([128, D], F32)
                nc.sync.dma_start(
                    out=x_t, in_=x_d.ap()[bi * TBLK + tt * 128:bi * TBLK + (tt + 1) * 128, :])
                s1 = stat.tile([128, 1], F32, tag="s1")
                nc.vector.tensor_reduce(s1, x_t, axis=mybir.AxisListType.X, op=OP.add)
                xn = lnpool.tile([128, D], F32, tag="xn")
                s2 = stat.tile([128, 1], F32, tag="s2")
                nc.vector.scalar_tensor_tensor(xn, x_t, 1.0, x_t, OP.mult, OP.mult,
                                               accum_out=s2)
                mu = stat.tile([128, 1], F32, tag="mu")
                nc.vector.tensor_scalar_mul(mu, s1, 1.0 / D)
                ex2 = stat.tile([128, 1], F32, tag="ex2")
                nc.vector.tensor_scalar_mul(ex2, s2, 1.0 / D)
                nmu = stat.tile([128, 1], F32, tag="nmu")
                nc.vector.tensor_scalar_mul(nmu, mu, -1.0)
                var = stat.tile([128, 1], F32, tag="var")
                nc.vector.scalar_tensor_tensor(var, nmu, mu, ex2, OP.mult, OP.add)
                sd = stat.tile([128, 1], F32, tag="sd")
                nc.scalar.activation(sd, var, AF.Sqrt, bias=c_eps)
                rstd = stat.tile([128, 1], F32, tag="rstd")
                nc.vector.reciprocal(rstd, sd)
                nc.vector.tensor_scalar(xn, x_t, mu, rstd, OP.subtract, OP.mult)
                for dt in range(4):
                    tp = tpsum.tile([128, 128], F32)
                    nc.tensor.transpose(tp, xn[:, dt * 128:(dt + 1) * 128], ident)
                    nc.vector.tensor_copy(out=xnt[dt][:, tt * 128:(tt + 1) * 128], in_=tp)

            # gamma/beta (in-place), clipped copy for the spline part
            zt = []
            for dt in range(4):
                nc.vector.tensor_scalar(
                    xnt[dt], xnt[dt], gam_sb[:, dt:dt + 1], bet_sb[:, dt:dt + 1],
                    OP.mult, OP.add)
                z = zpool.tile([128, TBLK], F32, tag=f"zx{dt}", name=f"z{dt}")
                nc.vector.tensor_scalar(z, xnt[dt], SCLIP, -SCLIP, OP.min, OP.max)
                zt.append(z)

            # ---- feature production + matmul accumulation (84 k-chunks) ----
            psum = [opsum.tile([128, TBLK], F32, tag=f"out{oc}", name=f"out{oc}") for oc in range(4)]
            kc = 0

            def consume(feat, w_sb, w_kc, fp32r):
                nonlocal kc
                for oc in range(4):
                    nc.tensor.matmul(psum[oc], w_sb[:, w_kc, oc * 128:(oc + 1) * 128],
                                     feat[:], start=(kc == 0), stop=(kc == N_KC - 1))
                kc += 1

            # base: relu(xn) @ w_b
            for dt in range(4):
                bf = fpool.tile([128, TBLK], BF16, tag="bfeat", bufs=1)
                nc.scalar.activation(bf, xnt[dt], AF.Relu)
                consume(bf, wb_sb, dt, False)

            # spline truncated powers: (s*relu(sgn*(z - a_q)))^3
            for q in range(NQ):
                sgn = FEAT_KNOTS[q][1]
                for dt in range(4):
                    u = spool.tile([128, TBLK], F32, tag="u")
                    nc.scalar.activation(u, zt[dt], AF.Relu,
                                         bias=c_knot[q], scale=sgn * S_CUBE)
                    r = fpool.tile([128, TBLK], F32R, tag="rfeat")
                    if q % 2 == 0:
                        nc.scalar.activation(r, u, AF.Square)
                    else:
                        nc.vector.scalar_tensor_tensor(r, u, 1.0, u, OP.mult, OP.mult)
                    nc.vector.scalar_tensor_tensor(r, r, 1.0, u, OP.mult, OP.mult)
                    consume(r, wr_sb, q * 4 + dt, True)

            # rbf: e_j = exp(-(x^2 - 2 g_j x)/den^2 - (g_j/den)^2)
            x2t = []
            for dt in range(4):
                x2 = zpool.tile([128, TBLK], F32, tag=f"zx{dt}", name=f"x2_{dt}")
                nc.vector.scalar_tensor_tensor(x2, xnt[dt], inv_den2, xnt[dt],
                                               OP.mult, OP.mult)
                x2t.append(x2)
            for j in range(NJ):
                for dt in range(4):
                    m = spool.tile([128, TBLK], F32, tag="u", name="m")
                    nc.vector.scalar_tensor_tensor(
                        m, xnt[dt], -2.0 * RBF_G[j] * inv_den2, x2t[dt],
                        OP.mult, OP.add)
                    e = fpool.tile([128, TBLK], BF16, tag="efeat")
                    nc.scalar.activation(e, m, AF.Exp, scale=-1.0,
                                         bias=c_rbf[j])
                    consume(e, we_sb, j * 4 + dt, False)
            assert kc == N_KC

            # ---- drain psum -> sbuf -> HBM ----
            for oc in range(4):
                ost = opool.tile([128, TBLK], F32, tag="ost")
                nc.vector.tensor_copy(out=ost, in_=psum[oc])
                nc.sync.dma_start(
                    out=out_d.ap()[oc * 128:(oc + 1) * 128, bi * TBLK:(bi + 1) * TBLK],
                    in_=ost)

    nc.finalize()
    _CACHED["nc"] = nc
    return nc


def _run(inputs: dict, trace: bool = False):
    x = np.asarray(inputs["x"], np.float32)
    gamma = np.asarray(inputs["ln_gamma"], np.float32)
    beta = np.asarray(inputs["ln_beta"], np.float32)
    w_b, w_r, w_e = _fold_weights(np.asarray(inputs["base_weight"], np.float32),
                                  np.asarray(inputs["spline_weight"], np.float32))
    xf = x.reshape(B * S, D)
    nc = _build_module()
    in_maps = []
    for c in range(N_CORES):
        in_maps.append({
            "x": np.ascontiguousarray(xf[c * TOK:(c + 1) * TOK]),
            "w_r": w_r, "w_e": w_e, "w_b": w_b,
            "gamma": gamma, "beta": beta,
        })
    res = run_bass_kernel_spmd(nc, in_maps, list(range(N_CORES)), trace=trace)
    outs = [res.results[c]["out"] for c in range(N_CORES)]       # [512, 2048] each
    full = np.concatenate(outs, axis=1)                          # [512, 16384]
    return np.ascontiguousarray(full.T).reshape(B, S, O).astype(np.float32), res


def kernel(**inputs) -> np.ndarray:
    out, _ = _run(inputs)
    return out


# revision 41
# speedup vs baseline: 7436.3923x; 7436.3923x over previous
"""BSRBF-KAN layer (LayerNorm + ReLU-base + B-spline+RBF spline matmul) on 8 trn2 cores.

Math:
  xn = LN(x) * gamma + beta
  base_out   = relu(xn) @ base_weight.T
  spline_out = (Bspline(xn) + RBF(xn)) @ spline_weight.T        (k = d*8 + j)
  out        = base_out + spline_out

Kernel strategy (data-parallel over the 16384 tokens, 2048/core):
  - Cubic B-splines on the uniform 12-knot grid are evaluated as 4th
    differences of truncated cubic powers of z = clamp(x, -3.5, 3.5):
        B_j = sum_i [1,-4,6,-4,1]_i * p_{j+i},  p_q = relu(+-(z - a_q))^3
    using LEFT powers (q=0..7) for B_0..3 and RIGHT powers (q=4..11) for
    B_4..7, which caps |feature| at ~66 and keeps the fold well-conditioned.
    The difference operator and 1/(6h^3) are folded into the spline weights
    on the host (w_r); the device computes 16 cube features per d.  The
    fold's cancellation requires fp32 features/weights -> the r-part matmul
    runs as float32r (full PE rate at N>=256).
  - RBF: e_j = exp(-((x-g_j)/den)^2) computed as exp(-(x^2 - 2 g_j x)/den^2
    - (g_j/den)^2): one fused DVE op + one ACT exp per basis -> bf16.
  - Output is produced o-major per core ([512, 2048]); host transposes.
"""

import math
import numpy as np
import ml_dtypes

import concourse.bacc as bacc
import concourse.bass as bass
import concourse.tile as tile
from concourse import mybir
from concourse.bass_utils import run_bass_kernel_spmd
from concourse.masks import make_identity
from contextlib import ExitStack

F32 = mybir.dt.float32
F32R = mybir.dt.float32r
BF16 = mybir.dt.bfloat16
AF = mybir.ActivationFunctionType
OP = mybir.AluOpType

# problem constants (hardcoded per contract)
B, S, D, O = 4, 4096, 512, 512
N_CORES = 8
TOK = (B * S) // N_CORES          # 2048 tokens per core
TBLK = 512                        # tokens per accumulation block (PSUM limit)
NBLK = TOK // TBLK                # 4 blocks per core
GRID_SIZE, SPLINE_ORDER = 5, 3
GRID_MIN, GRID_MAX = -1.5, 1.5
H = (GRID_MAX - GRID_MIN) / GRID_SIZE                    # 0.6
KNOTS = [(-SPLINE_ORDER + i) * H + GRID_MIN for i in range(12)]   # a_0..a_11
# mixed truncated-power basis: B_0..3 from LEFT powers l_q=relu(a_q-z)^3 (q=0..7),
# B_4..7 from RIGHT powers r_q=relu(z-a_q)^3 (q=4..11). Caps |feature| at ~66.
FEAT_KNOTS = [(KNOTS[q], -1.0) for q in range(8)] + [(KNOTS[q], +1.0) for q in range(4, 12)]
NQ = 16                          # truncated-power features
NJ = 8                           # rbf / spline bases
SCLIP = 3.5                      # beyond all supports; B == 0 there (both sides)
S_CUBE = (1.0 / (6.0 * H ** 3)) ** (1.0 / 3.0)           # folded 1/(6h^3)
RBF_DEN = (GRID_MAX - GRID_MIN) / (NJ - 1)               # 3/7
RBF_G = [GRID_MIN + i * RBF_DEN for i in range(NJ)]
LN_EPS = 1e-5

N_KC = 4 + NQ * 4 + NJ * 4       # 100 k-chunks of 128: base, r, e


def _fold_weights(base_weight: np.ndarray, spline_weight: np.ndarray):
    """Host-side weight prep. Returns (w_b [512,512] bf16, w_r [8192,512] f32,
    w_e [4096,512] bf16), all in lhsT layout [k, o]."""
    Wjd = spline_weight.reshape(O, D, NJ).astype(np.float64)   # [o, d, j]
    c = np.array([1.0, -4.0, 6.0, -4.0, 1.0])
    w_r = np.zeros((NQ, D, O), np.float64)                     # [q, d, o]
    for q in range(8):            # left features serve B_0..B_3
        for i in range(5):
            j = q - i
            if 0 <= j <= 3:
                w_r[q] += c[i] * Wjd[:, :, j].T
    w_r[:8] *= -1.0               # left feature = min(z-a,0)^3 = -(relu(a-z))^3
    for qi, q in enumerate(range(4, 12)):   # right features serve B_4..B_7
        for i in range(5):
            j = q - i
            if 4 <= j <= 7:
                w_r[8 + qi] += c[i] * Wjd[:, :, j].T
    w_r = (w_r * (1.0 / (6.0 * H ** 3))).reshape(NQ * D, O).astype(np.float32)
    w_e = np.ascontiguousarray(Wjd.transpose(2, 1, 0)).reshape(NJ * D, O)
    w_e = w_e.astype(ml_dtypes.bfloat16)
    w_b = np.ascontiguousarray(base_weight.T).astype(ml_dtypes.bfloat16)
    return w_b, w_r, w_e


_CACHED = {}


def _build_module(repeats: int = 1):
    key = ("nc", repeats)
    if key in _CACHED:
        return _CACHED[key]
    nc = bacc.Bacc("TRN2", target_bir_lowering=False, debug=False,
                   num_devices=N_CORES)
    x_d = nc.dram_tensor("x", [TOK, D], F32, kind="ExternalInput")
    wr_d = nc.dram_tensor("w_r", [NQ * D, O], F32R, kind="ExternalInput")
    we_d = nc.dram_tensor("w_e", [NJ * D, O], BF16, kind="ExternalInput")
    wb_d = nc.dram_tensor("w_b", [D, O], BF16, kind="ExternalInput")
    g_d = nc.dram_tensor("gamma", [D], F32, kind="ExternalInput")
    be_d = nc.dram_tensor("beta", [D], F32, kind="ExternalInput")
    out_d = nc.dram_tensor("out", [O, TOK], F32, kind="ExternalOutput")

    inv_den2 = 1.0 / (RBF_DEN * RBF_DEN)

    with tile.TileContext(nc) as tc, ExitStack() as ctx:
        wpool = ctx.enter_context(tc.tile_pool(name="weights", bufs=1))
        xpool = ctx.enter_context(tc.tile_pool(name="xin", bufs=2))
        lnpool = ctx.enter_context(tc.tile_pool(name="ln", bufs=2))
        stat = ctx.enter_context(tc.tile_pool(name="stat", bufs=2))
        xtp = ctx.enter_context(tc.tile_pool(name="xnT", bufs=1))
        zpool = ctx.enter_context(tc.tile_pool(name="zt", bufs=1))
        fpool = ctx.enter_context(tc.tile_pool(name="feat", bufs=3))
        spool = ctx.enter_context(tc.tile_pool(name="scratch", bufs=2))
        opool = ctx.enter_context(tc.tile_pool(name="ostage", bufs=1))
        tpsum = ctx.enter_context(tc.tile_pool(name="tpsum", bufs=4, space="PSUM"))
        opsum = ctx.enter_context(tc.tile_pool(name="opsum", bufs=1, space="PSUM"))

        # ---- resident weights (chunked DMAs so first matmuls start early) ----
        wr_ap = wr_d.ap().rearrange("(c p) o -> p c o", p=128)
        we_ap = we_d.ap().rearrange("(c p) o -> p c o", p=128)
        wb_ap = wb_d.ap().rearrange("(c p) o -> p c o", p=128)
        wr_sb = wpool.tile([128, NQ * 4, O], F32R)
        we_sb = wpool.tile([128, NJ * 4, O], BF16)
        wb_sb = wpool.tile([128, 4, O], BF16)
        def emit_weight_dmas():
            nc.sync.dma_start(out=wb_sb, in_=wb_ap)
            for dt in range(4):
                sl = slice(dt, dt + 29, 4)
                nc.sync.dma_start(out=we_sb[:, sl], in_=we_ap[:, sl])
            for dt in range(4):
                for qb in range(2):
                    # 8 q-chunks (stride 4 in chunk index), 2MB per DMA
                    sl = slice(qb * 32 + dt, qb * 32 + dt + 29, 4)
                    nc.sync.dma_start(out=wr_sb[:, sl], in_=wr_ap[:, sl])
        gam_sb = wpool.tile([128, 4], F32)
        nc.sync.dma_start(out=gam_sb, in_=g_d.ap().rearrange("(c p) -> p c", p=128))
        bet_sb = wpool.tile([128, 4], F32)
        nc.sync.dma_start(out=bet_sb, in_=be_d.ap().rearrange("(c p) -> p c", p=128))
        ident = wpool.tile([128, 128], F32)
        make_identity(nc, ident)

        # ACT bias constants must live in SBUF ([128,1] per-partition APs)
        bias_vals = ([LN_EPS]
                     + [-sgn * a * S_CUBE for (a, sgn) in FEAT_KNOTS]
                     + [-(RBF_G[j] ** 2) * inv_den2 for j in range(NJ)])
        consts = wpool.tile([128, len(bias_vals)], F32)
        for i, v in enumerate(bias_vals):
            nc.gpsimd.memset(consts[:, i:i + 1], v)
        c_eps = consts[:, 0:1]
        c_knot = [consts[:, 1 + q:2 + q] for q in range(NQ)]
        c_rbf = [consts[:, 1 + NQ + j:2 + NQ + j] for j in range(NJ)]

        for bi_rep in range(NBLK * repeats):
            bi = bi_rep % NBLK
            # ---- LayerNorm (token-major) + transpose to [d, tok] ----
            xnt = [xtp.tile([128, TBLK], F32, tag=f"xnt{dt}", name=f"xnt{dt}") for dt in range(4)]
            xts = []
            for tt in range(TBLK // 128):
                x_t = xpool.tile([128, D], F32, bufs=4)
                nc.sync.dma_start(
                    out=x_t, in_=x_d.ap()[bi * TBLK + tt * 128:bi * TBLK + (tt + 1) * 128, :])
                st6 = stat.tile([128, nc.vector.BN_STATS_DIM], F32, tag="st6")
                nc.vector.bn_stats(out=st6, in_=x_t)
                mv = stat.tile([128, nc.vector.BN_AGGR_DIM], F32, tag="mv")
                nc.vector.bn_aggr(out=mv, in_=st6)
                sd = stat.tile([128, 1], F32, tag="sd")
                nc.scalar.activation(sd, mv[:, 1:2], AF.Sqrt, bias=c_eps)
                rstd = stat.tile([128, 1], F32, tag="rstd")
                nc.vector.reciprocal(rstd, sd)
                nc.vector.tensor_scalar(x_t, x_t, mv[:, 0:1], rstd,
                                        OP.subtract, OP.mult)
                xts.append(x_t)

            if bi_rep == 0:
                emit_weight_dmas()

            psum = [opsum.tile([128, TBLK], F32, tag=f"out{oc}", name=f"out{oc}") for oc in range(4)]
            kc = 0

            def consume(feat, w_sb, w_kc, fp32r):
                nonlocal kc
                for oc in range(4):
                    nc.tensor.matmul(psum[oc], w_sb[:, w_kc, oc * 128:(oc + 1) * 128],
                                     feat[:], start=(kc == 0), stop=(kc == N_KC - 1))
                kc += 1

            # transpose dt-major; finalize each dt (gamma/beta, base feature,
            # clip) as soon as its 4 transposes land, then issue the base MMs
            zt, bft = [], []
            for dt in range(4):
                for tt in range(TBLK // 128):
                    tp = tpsum.tile([128, 128], F32)
                    nc.tensor.transpose(tp, xts[tt][:, dt * 128:(dt + 1) * 128], ident)
                    nc.vector.tensor_copy(out=xnt[dt][:, tt * 128:(tt + 1) * 128], in_=tp)
                nc.vector.tensor_scalar(
                    xnt[dt], xnt[dt], gam_sb[:, dt:dt + 1], bet_sb[:, dt:dt + 1],
                    OP.mult, OP.add)
                bf = fpool.tile([128, TBLK], BF16, tag="bfeat", bufs=1, name=f"bf{dt}")
                nc.vector.tensor_scalar_max(bf, xnt[dt], 0.0)
                bft.append(bf)
                z = zpool.tile([128, TBLK], F32, tag=f"zx{dt}", name=f"z{dt}")
                nc.gpsimd.tensor_scalar(z, xnt[dt], SCLIP, -SCLIP, OP.min, OP.max)
                zt.append(z)
                consume(bf, wb_sb, dt, False)

            # spline truncated powers u^3, u = relu(sgn*(z - a_q)); 1/(6h^3) is
            # folded into w_r on the host
            # rbf: e_j = exp(-(x^2 - 2 g_j x)/den^2 - (g_j/den)^2)
            for dt in range(4):
                x2 = spool.tile([128, TBLK], F32, tag="u", name=f"x2_{dt}", bufs=3)
                nc.gpsimd.tensor_tensor(out=x2, in0=xnt[dt], in1=xnt[dt], op=OP.mult)
                for j in range(NJ):
                    m = spool.tile([128, TBLK], F32, tag="u", name="m", bufs=3)
                    nc.vector.scalar_tensor_tensor(
                        m, xnt[dt], -2.0 * RBF_G[j], x2, OP.mult, OP.add)
                    e = fpool.tile([128, TBLK], BF16, tag="efeat")
                    nc.scalar.activation(e, m, AF.Exp, scale=-inv_den2,
                                         bias=c_rbf[j])
                    consume(e, we_sb, j * 4 + dt, False)
            for dt in range(4):
                for q in range(NQ):
                    a, sgn = FEAT_KNOTS[q]
                    u = spool.tile([128, TBLK], F32, tag="u", bufs=3)
                    op1 = OP.max if sgn > 0 else OP.min
                    nc.vector.tensor_scalar(u, zt[dt], -a, 0.0, OP.add, op1)
                    r = fpool.tile([128, TBLK], F32R, tag="rfeat", bufs=3)
                    nc.scalar.activation(r, u, AF.Square)
                    if (q * 4 + dt) % 3 == 0:
                        nc.vector.tensor_tensor(out=r, in0=r, in1=u, op=OP.mult)
                    else:
                        nc.gpsimd.tensor_tensor(out=r, in0=r, in1=u, op=OP.mult)
                    consume(r, wr_sb, q * 4 + dt, True)
            assert kc == N_KC

            # ---- drain psum -> sbuf -> HBM ----
            for oc in range(4):
                ost = opool.tile([128, TBLK], F32, tag="ost")
                nc.scalar.copy(out=ost, in_=psum[oc])
                nc.gpsimd.dma_start(
                    out=out_d.ap()[oc * 128:(oc + 1) * 128, bi * TBLK:(bi + 1) * TBLK],
                    in_=ost)

    nc.finalize()
    _CACHED[key] = nc
    return nc


def _run(inputs: dict, trace: bool = False):
    x = np.asarray(inputs["x"], np.float32)
    gamma = np.asarray(inputs["ln_gamma"], np.float32)
    beta = np.asarray(inputs["ln_beta"], np.float32)
    w_b, w_r, w_e = _fold_weights(np.asarray(inputs["base_weight"], np.float32),
                                  np.asarray(inputs["spline_weight"], np.float32))
    xf = x.reshape(B * S, D)
    nc = _build_module()
    in_maps = []
    for c in range(N_CORES):
        in_maps.append({
            "x": np.ascontiguousarray(xf[c * TOK:(c + 1) * TOK]),
            "w_r": w_r, "w_e": w_e, "w_b": w_b,
            "gamma": gamma, "beta": beta,
        })
    res = run_bass_kernel_spmd(nc, in_maps, list(range(N_CORES)), trace=trace)
    outs = [res.results[c]["out"] for c in range(N_CORES)]       # [512, 2048] each
    full = np.concatenate(outs, axis=1)                          # [512, 16384]
    return np.ascontiguousarray(full.T).reshape(B, S, O).astype(np.float32), res


def kernel(**inputs) -> np.ndarray:
    out, _ = _run(inputs)
    return out
